# revision 8
# baseline (speedup 1.0000x reference)
"""MeshGraphEncoder Trainium2 kernel (8 NeuronCores, SPMD).

Strategy:
  * Host: sort edges by dst mesh node; shard edges + mesh nodes across 8
    cores by disjoint dst-node ranges (5120 nodes/core) -> per-core
    segment sums are complete, NO collective needed. Grid nodes are
    sharded data-parallel (16384/core) for the residual src MLP.
  * Host gathers the edge-MLP input [efeat | grid[src] | mesh[dst]] per
    core in feature-major (transposed) layout so the device runs dense
    weight-stationary matmuls (this is the "shard edges and their
    gathered src/dst features" layout from the sharding hint).
  * Device per core: edge MLP (3-chunk K=384 matmul, SiLU, second matmul
    with activation-slices as the stationary operand -> edge-major
    output, LayerNorm via per-partition stats), then a windowed
    segment-sum: selection-matrix matmuls accumulate edge messages into
    a per-128-node-window PSUM tile (edges arrive sorted, so each window
    accumulates over a short run of tiles). Window results feed the dst
    MLP; LN scale/shift of the edge MLP are folded into the dst MLP
    weights (segment-sum is linear; per-node edge counts carry the bias
    term as a rank-1 matmul). The grid MLP runs the same block pipeline.
  * All stages are emitted software-pipelined (3-stage stagger) so
    PE / ACT / DVE / DMA overlap across 512-edge blocks.
"""
import sys

sys.path.insert(0, "/opt/trn_rl_repo")

import numpy as np

NG, NM, E, D = 131072, 40962, 524288, 128
P = 128
NCORES = 8
NPC = 5120            # mesh nodes per core (core 7 takes the +2 remainder)
NWIN = 41             # 128-node windows per core (40*128=5120; win 40: +2)
NPAD = NWIN * P       # padded node slots per core = 5248
NGC = NG // NCORES    # grid rows per core = 16384
BLK = 4               # tiles per block (512 edges)
EPS = 1e-5


# ---------------------------------------------------------------- host prep
def _plan(dst):
    """Edge->core/window assignment and uniform tile capacities."""
    order = np.argsort(dst, kind="stable")
    dst_s = dst[order]
    core_lo = np.searchsorted(dst_s, np.arange(NCORES) * NPC)
    core_hi = np.append(core_lo[1:], E)

    # per (core, window) counts
    cnt = np.zeros((NCORES, NWIN), np.int64)
    for c in range(NCORES):
        loc = dst_s[core_lo[c]:core_hi[c]] - c * NPC
        cnt[c] = np.bincount(loc // P, minlength=NWIN)
    T = np.maximum(1, -(-cnt.max(axis=0) // P))      # tiles per window
    NT = int(T.sum())
    if NT % BLK:
        T[-1] += BLK - NT % BLK
        NT = int(T.sum())
    wstart_tile = np.concatenate([[0], np.cumsum(T)])[:-1]
    tile_to_win = np.repeat(np.arange(NWIN), T)
    return order, core_lo, core_hi, cnt, T, NT, wstart_tile, tile_to_win


def _prep_core(c, order, core_lo, core_hi, cnt, T, NT, wstart_tile,
               g2m_efeat, grid_nfeat, mesh_nfeat, src, dst):
    EPAD = NT * P
    eids = np.full(EPAD, -1, np.int64)
    for w in range(NWIN):
        lo = core_lo[c] + int(cnt[c, :w].sum())
        n = int(cnt[c, w])
        s = int(wstart_tile[w]) * P
        eids[s:s + n] = order[lo:lo + n]
    mask = eids >= 0
    eid = eids[mask]

    X = np.zeros((EPAD, 3, D), np.float32)
    X[mask, 0] = g2m_efeat[eid]
    X[mask, 1] = grid_nfeat[src[eid]]
    X[mask, 2] = mesh_nfeat[dst[eid]]
    xcatT = np.ascontiguousarray(X.transpose(1, 2, 0))      # [3, D, EPAD]
    del X

    slots = np.full(EPAD, -1.0, np.float32)
    slots[mask] = ((dst[eid] - c * NPC) % P).astype(np.float32)
    slotsT = np.ascontiguousarray(slots.reshape(NT, P).T)   # [P, NT]

    n_real = min(NM - c * NPC, NPAD)
    meshpad = np.zeros((NPAD, D), np.float32)
    meshpad[:n_real] = mesh_nfeat[c * NPC:c * NPC + n_real]
    meshT = np.ascontiguousarray(meshpad.T)                 # [D, NPAD]
    cnt_row = np.bincount(dst[eid] - c * NPC,
                          minlength=NPAD).astype(np.float32)[None, :]

    gridT = np.ascontiguousarray(grid_nfeat[c * NGC:(c + 1) * NGC].T)
    return dict(xcatT=xcatT, slotsT=slotsT, meshT=meshT, cnt_row=cnt_row,
                meshpad=meshpad, gridT=gridT)


def _bcast(v):
    return np.ascontiguousarray(np.broadcast_to(
        np.asarray(v, np.float32).reshape(1, D), (P, D)))


# ---------------------------------------------------------------- device IR
def _emit_ln(nc, tc, pools, y2_ps, b1b_sb, eps_sb, mybir, nt=BLK):
    """Edge/node-major LayerNorm on a [P, nt, D] PSUM tile view.

    Returns xn (normalized, un-scaled) in a fresh SBUF tile."""
    f32 = mybir.dt.float32
    AL = mybir.AluOpType
    AF = mybir.ActivationFunctionType
    y2_sb = pools["y2sb"].tile([P, BLK, D], f32)
    sumsq = pools["st_sq"].tile([P, BLK], f32)
    sq_scr = pools["sq_scr"].tile([P, D], f32)
    for t in range(nt):
        nc.vector.tensor_tensor(out=y2_sb[:, t, :], in0=y2_ps[:, t, :],
                                in1=b1b_sb[:], op=AL.add)
        nc.scalar.activation(sq_scr[:], y2_sb[:, t, :], AF.Square,
                             accum_out=sumsq[:, t:t + 1])
    sums = pools["st_s"].tile([P, BLK], f32)
    nc.vector.tensor_reduce(out=sums[:, :nt], in_=y2_sb[:, :nt, :],
                            axis=mybir.AxisListType.X, op=AL.add)
    mu = pools["st_mu"].tile([P, BLK], f32)
    nc.vector.tensor_scalar_mul(mu[:, :nt], sums[:, :nt], 1.0 / D)
    musq = pools["st_m2"].tile([P, BLK], f32)
    nc.vector.tensor_tensor(musq[:, :nt], mu[:, :nt], mu[:, :nt], op=AL.mult)
    var = pools["st_v"].tile([P, BLK], f32)
    nc.vector.scalar_tensor_tensor(out=var[:, :nt], in0=sumsq[:, :nt],
                                   scalar=1.0 / D, in1=musq[:, :nt],
                                   op0=AL.mult, op1=AL.subtract)
    std = pools["st_sd"].tile([P, BLK], f32)
    nc.scalar.activation(std[:, :nt], var[:, :nt], AF.Sqrt,
                         bias=eps_sb[:, :1])
    rstd = pools["st_r"].tile([P, BLK], f32)
    nc.vector.reciprocal(rstd[:, :nt], std[:, :nt])
    xn = pools["xn"].tile([P, BLK, D], f32)
    for t in range(nt):
        nc.vector.tensor_scalar(out=xn[:, t, :], in0=y2_sb[:, t, :],
                                scalar1=mu[:, t:t + 1],
                                scalar2=rstd[:, t:t + 1],
                                op0=AL.subtract, op1=AL.mult)
    return xn


def _build(meta):
    import concourse.bacc as bacc
    import concourse.tile as tile
    from concourse import mybir

    f32 = mybir.dt.float32
    AL = mybir.AluOpType
    AF = mybir.ActivationFunctionType
    NT = meta["NT"]
    EPAD = NT * P
    NBLK = NT // BLK
    T = meta["T"]
    wstart_tile = meta["wstart_tile"]
    tile_to_win = meta["tile_to_win"]
    wstart = {w: int(wstart_tile[w]) for w in range(NWIN)}
    wend = {w: int(wstart_tile[w] + T[w] - 1) for w in range(NWIN)}

    nc = bacc.Bacc("TRN2", target_bir_lowering=False, debug=False,
                   num_devices=NCORES)

    def din(name, shape):
        return nc.dram_tensor(name, shape, f32, kind="ExternalInput")

    xcatT = din("xcatT", [3, P, EPAD])
    slotsT = din("slotsT", [P, NT])
    meshT = din("meshT", [P, NPAD])
    cnt_row = din("cnt_row", [1, NPAD])
    mesh_res = din("mesh_res", [NPAD, D])
    gridT = din("gridT", [P, NGC])
    grid_res = din("grid_res", [NGC, D])
    w0e = din("w0e", [P, 3, D])
    w1e = din("w1e", [P, D])
    b0e = din("b0e", [P, 1])
    b1e_b = din("b1e_b", [P, D])
    w0d = din("w0d", [P, 2, D])
    w1d = din("w1d", [P, D])
    b0d = din("b0d", [P, 1])
    b1d_b = din("b1d_b", [P, D])
    gd_b = din("gd_b", [P, D])
    btW0 = din("btW0", [1, D])
    w0s = din("w0s", [P, D])
    w1s = din("w1s", [P, D])
    b0s = din("b0s", [P, 1])
    b1s_b = din("b1s_b", [P, D])
    gs_b = din("gs_b", [P, D])
    iota = din("iota", [P, P])
    mesh_out = nc.dram_tensor("mesh_out", [NPAD, D], f32,
                              kind="ExternalOutput")
    grid_out = nc.dram_tensor("grid_out", [NGC, D], f32,
                              kind="ExternalOutput")

    with tile.TileContext(nc) as tc:
        import contextlib
        ctx = contextlib.ExitStack()
        with ctx:
            cp = ctx.enter_context(tc.tile_pool(name="const", bufs=1))
            pools = {}
            for name, bufs in [("xcat", 3), ("h", 3), ("y2sb", 3), ("xn", 3),
                               ("sel", 8), ("slots", 3), ("sq_scr", 3),
                               ("st_sq", 3), ("st_s", 3), ("st_mu", 3),
                               ("st_m2", 3), ("st_v", 3), ("st_sd", 3),
                               ("st_r", 3), ("aggb", 2), ("msT", 2),
                               ("cnt", 2), ("res", 2), ("fin", 2),
                               ("outt", 2)]:
                pools[name] = ctx.enter_context(
                    tc.tile_pool(name=name, bufs=bufs))
            pp_h = ctx.enter_context(
                tc.tile_pool(name="pp_h", bufs=2, space="PSUM"))
            pp_y2 = ctx.enter_context(
                tc.tile_pool(name="pp_y2", bufs=2, space="PSUM"))
            pp_agg = ctx.enter_context(
                tc.tile_pool(name="pp_agg", bufs=3, space="PSUM"))

            def cload(ap, shape, name):
                t = cp.tile(shape, f32, name=name, tag=name)
                nc.sync.dma_start(out=t[:], in_=ap[:])
                return t

            w0e_sb = cload(w0e.ap().rearrange("p c d -> p (c d)"),
                           [P, 3 * D], "w0e_sb")
            w0e_sb = w0e_sb[:].rearrange("p (c d) -> p c d", c=3)
            w1e_sb = cload(w1e, [P, D], "w1e_sb")
            b0e_sb = cload(b0e, [P, 1], "b0e_sb")
            b1e_sb = cload(b1e_b, [P, D], "b1e_sb")
            w0d_sb = cload(w0d.ap().rearrange("p c d -> p (c d)"),
                           [P, 2 * D], "w0d_sb")
            w0d_sb = w0d_sb[:].rearrange("p (c d) -> p c d", c=2)
            w1d_sb = cload(w1d, [P, D], "w1d_sb")
            b0d_sb = cload(b0d, [P, 1], "b0d_sb")
            b1d_sb = cload(b1d_b, [P, D], "b1d_sb")
            gd_sb = cload(gd_b, [P, D], "gd_sb")
            btW0_sb = cload(btW0, [1, D], "btW0_sb")
            w0s_sb = cload(w0s, [P, D], "w0s_sb")
            w1s_sb = cload(w1s, [P, D], "w1s_sb")
            b0s_sb = cload(b0s, [P, 1], "b0s_sb")
            b1s_sb = cload(b1s_b, [P, D], "b1s_sb")
            gs_sb = cload(gs_b, [P, D], "gs_sb")
            iota_sb = cload(iota, [P, P], "iota_sb")
            eps_sb = cp.tile([P, 1], f32)
            nc.vector.memset(eps_sb[:], EPS)

            # ---------------- phase A/B: edge pipeline + mesh MLP ---------
            stA = {}   # block -> (h_sb,)
            stB = {}   # block -> (xn, sels)
            aggps = {}  # window -> psum tile
            aggb = {}   # batch -> sbuf tile [P, 4*P]

            def stageA(b):
                off = b * BLK * P
                xcat = pools["xcat"].tile([P, 3, BLK * P], f32)
                nc.sync.dma_start(
                    out=xcat[:],
                    in_=xcatT.ap()[:, :, off:off + BLK * P]
                    .rearrange("c p e -> p c e"))
                slots = pools["slots"].tile([P, BLK], f32)
                nc.sync.dma_start(out=slots[:],
                                  in_=slotsT.ap()[:, b * BLK:(b + 1) * BLK])
                h_ps = pp_h.tile([P, BLK * P], f32)
                for cch in range(3):
                    nc.tensor.matmul(h_ps[:], w0e_sb[:, cch, :],
                                     xcat[:, cch, :],
                                     start=(cch == 0), stop=(cch == 2))
                h_sb = pools["h"].tile([P, BLK * P], f32)
                nc.scalar.activation(h_sb[:], h_ps[:], AF.Silu,
                                     bias=b0e_sb[:, :1])
                stA[b] = (h_sb, slots)

            def stageB(b):
                h_sb, slots = stA.pop(b)
                y2_ps = pp_y2.tile([P, BLK, D], f32)
                for t in range(BLK):
                    nc.tensor.matmul(y2_ps[:, t, :],
                                     h_sb[:, t * P:(t + 1) * P], w1e_sb[:],
                                     start=True, stop=True)
                xn = _emit_ln(nc, tc, pools, y2_ps, b1e_sb, eps_sb, mybir)
                sels = []
                for t in range(BLK):
                    sel = pools["sel"].tile([P, P], f32)
                    nc.vector.tensor_tensor(
                        sel[:], slots[:, t:t + 1].to_broadcast([P, P]),
                        iota_sb[:], op=AL.is_equal)
                    sels.append(sel)
                stB[b] = (xn, sels)

            def phaseB(batch):
                """dst/mesh MLP over up to 4 windows (512 node slots)."""
                nb = batch * 4 * P
                nwt = min(4, NWIN - batch * 4)   # node tiles this batch
                nw = nwt * P
                ab = aggb.pop(batch)
                msT = pools["msT"].tile([P, 4 * P], f32)
                nc.sync.dma_start(out=msT[:, :nw],
                                  in_=meshT.ap()[:, nb:nb + nw])
                cnt = pools["cnt"].tile([1, 4 * P], f32)
                nc.sync.dma_start(out=cnt[:, :nw],
                                  in_=cnt_row.ap()[:, nb:nb + nw])
                h_ps = pp_h.tile([P, BLK * P], f32)
                nc.tensor.matmul(h_ps[:, :nw], w0d_sb[:, 0, :], ab[:, :nw],
                                 start=True, stop=False)
                nc.tensor.matmul(h_ps[:, :nw], w0d_sb[:, 1, :], msT[:, :nw],
                                 start=False, stop=False)
                nc.tensor.matmul(h_ps[:, :nw], btW0_sb[:1, :], cnt[:1, :nw],
                                 start=False, stop=True,
                                 skip_group_check=True)
                h_sb = pools["h"].tile([P, BLK * P], f32)
                nc.scalar.activation(h_sb[:, :nw], h_ps[:, :nw], AF.Silu,
                                     bias=b0d_sb[:, :1])
                y2_ps = pp_y2.tile([P, BLK, D], f32)
                for t in range(nwt):
                    nc.tensor.matmul(y2_ps[:, t, :],
                                     h_sb[:, t * P:(t + 1) * P], w1d_sb[:],
                                     start=True, stop=True)
                xn = _emit_ln(nc, tc, pools, y2_ps, b1d_sb, eps_sb, mybir,
                              nt=nwt)
                fin = pools["fin"].tile([P, BLK, D], f32)
                nc.vector.tensor_tensor(
                    fin[:, :nwt, :], xn[:, :nwt, :],
                    gd_sb[:][:, None, :].to_broadcast([P, nwt, D]),
                    op=AL.mult)
                res = pools["res"].tile([P, BLK, D], f32)
                nc.sync.dma_start(
                    out=res[:, :nwt, :],
                    in_=mesh_res.ap()[nb:nb + nw, :]
                    .rearrange("(t p) d -> p t d", p=P))
                out = pools["outt"].tile([P, BLK, D], f32)
                nc.vector.tensor_tensor(out[:, :nwt, :], fin[:, :nwt, :],
                                        res[:, :nwt, :], op=AL.add)
                nc.sync.dma_start(
                    out=mesh_out.ap()[nb:nb + nw, :]
                    .rearrange("(t p) d -> p t d", p=P),
                    in_=out[:, :nwt, :])

            def stageC(b):
                xn, sels = stB.pop(b)
                for t in range(BLK):
                    gt = b * BLK + t
                    w = int(tile_to_win[gt])
                    if gt == wstart[w]:
                        aggps[w] = pp_agg.tile([P, P], f32, name="aggw", tag="aggw")
                    nc.tensor.matmul(aggps[w][:], xn[:, t, :], sels[t][:],
                                     start=(gt == wstart[w]),
                                     stop=(gt == wend[w]),
                                     skip_group_check=True)
                    if gt == wend[w]:
                        batch, col = w // 4, w % 4
                        if col == 0:
                            aggb[batch] = pools["aggb"].tile([P, 4 * P], f32,
                                                             name="aggb", tag="aggb")
                            if batch == NWIN // 4:   # partial last batch
                                nc.vector.memset(aggb[batch][:], 0.0)
                        nc.scalar.copy(aggb[batch][:, col * P:(col + 1) * P],
                                       aggps.pop(w)[:])
                        if col == 3 or w == NWIN - 1:
                            phaseB(batch)

            for b in range(NBLK + 2):
                if b < NBLK:
                    stageA(b)
                if 1 <= b <= NBLK:
                    stageB(b - 1)
                if 2 <= b:
                    stageC(b - 2)

            # ---------------- phase C: grid MLP ---------------------------
            NCB = NGC // (BLK * P)   # 32 blocks
            stG = {}

            def gridA(b):
                rb = b * BLK * P
                gT = pools["msT"].tile([P, BLK * P], f32, tag="msT")
                nc.sync.dma_start(out=gT[:], in_=gridT.ap()[:, rb:rb + BLK * P])
                h_ps = pp_h.tile([P, BLK * P], f32)
                nc.tensor.matmul(h_ps[:], w0s_sb[:], gT[:],
                                 start=True, stop=True)
                h_sb = pools["h"].tile([P, BLK * P], f32)
                nc.scalar.activation(h_sb[:], h_ps[:], AF.Silu,
                                     bias=b0s_sb[:, :1])
                stG[b] = h_sb

            def gridB(b):
                rb = b * BLK * P
                h_sb = stG.pop(b)
                y2_ps = pp_y2.tile([P, BLK, D], f32)
                for t in range(BLK):
                    nc.tensor.matmul(y2_ps[:, t, :],
                                     h_sb[:, t * P:(t + 1) * P], w1s_sb[:],
                                     start=True, stop=True)
                xn = _emit_ln(nc, tc, pools, y2_ps, b1s_sb, eps_sb, mybir)
                fin = pools["fin"].tile([P, BLK, D], f32)
                nc.vector.tensor_tensor(
                    fin[:], xn[:],
                    gs_sb[:][:, None, :].to_broadcast([P, BLK, D]), op=AL.mult)
                res = pools["res"].tile([P, BLK, D], f32)
                nc.sync.dma_start(
                    out=res[:],
                    in_=grid_res.ap()[rb:rb + BLK * P, :]
                    .rearrange("(t p) d -> p t d", p=P))
                out = pools["outt"].tile([P, BLK, D], f32)
                nc.vector.tensor_tensor(out[:], fin[:], res[:], op=AL.add)
                nc.sync.dma_start(
                    out=grid_out.ap()[rb:rb + BLK * P, :]
                    .rearrange("(t p) d -> p t d", p=P),
                    in_=out[:])

            for b in range(NCB + 1):
                if b < NCB:
                    gridA(b)
                if b >= 1:
                    gridB(b - 1)

    nc.compile()
    return nc


# ---------------------------------------------------------------- kernel()
_CACHE = {}


def prepare(g2m_efeat, grid_nfeat, mesh_nfeat, src_idx, dst_idx,
            edge_W0, edge_b0, edge_W1, edge_b1, edge_g, edge_bt,
            src_W0, src_b0, src_W1, src_b1, src_g, src_bt,
            dst_W0, dst_b0, dst_W1, dst_b1, dst_g, dst_bt):
    g2m_efeat = np.asarray(g2m_efeat, np.float32)
    grid_nfeat = np.asarray(grid_nfeat, np.float32)
    mesh_nfeat = np.asarray(mesh_nfeat, np.float32)
    src = np.asarray(src_idx).astype(np.int64)
    dst = np.asarray(dst_idx).astype(np.int64)

    order, core_lo, core_hi, cnt, T, NT, wstart_tile, tile_to_win = _plan(dst)

    meta = dict(NT=NT, T=T, wstart_tile=wstart_tile, tile_to_win=tile_to_win)
    key = (NT, tuple(T.tolist()))
    if key not in _CACHE:
        _CACHE[key] = _build(meta)
    nc = _CACHE[key]

    consts = dict(
        w0e=np.ascontiguousarray(
            np.asarray(edge_W0, np.float32).reshape(3, P, D).transpose(1, 0, 2)),
        w1e=np.asarray(edge_W1, np.float32),
        b0e=np.asarray(edge_b0, np.float32).reshape(P, 1),
        b1e_b=_bcast(edge_b1),
        w0d=np.ascontiguousarray(np.stack(
            [np.asarray(dst_W0, np.float32)[:D] *
             np.asarray(edge_g, np.float32)[:, None],
             np.asarray(dst_W0, np.float32)[D:]], 0).transpose(1, 0, 2)),
        w1d=np.asarray(dst_W1, np.float32),
        b0d=np.asarray(dst_b0, np.float32).reshape(P, 1),
        b1d_b=_bcast(dst_b1),
        gd_b=_bcast(dst_g),
        btW0=(np.asarray(edge_bt, np.float32) @
              np.asarray(dst_W0, np.float32)[:D]).reshape(1, D),
        w0s=np.asarray(src_W0, np.float32),
        w1s=np.asarray(src_W1, np.float32),
        b0s=np.asarray(src_b0, np.float32).reshape(P, 1),
        b1s_b=_bcast(src_b1),
        gs_b=_bcast(src_g),
        iota=np.ascontiguousarray(np.broadcast_to(
            np.arange(P, dtype=np.float32)[None, :], (P, P))),
    )

    in_maps = []
    for c in range(NCORES):
        d = _prep_core(c, order, core_lo, core_hi, cnt, T, NT, wstart_tile,
                       g2m_efeat, grid_nfeat, mesh_nfeat, src, dst)
        mesh_res = d.pop("meshpad") + np.asarray(dst_bt, np.float32)[None, :]
        grid_res = (grid_nfeat[c * NGC:(c + 1) * NGC] +
                    np.asarray(src_bt, np.float32)[None, :])
        in_maps.append(dict(**d, mesh_res=mesh_res, grid_res=grid_res,
                            **consts))

    return nc, in_maps


def kernel(**inputs):
    from concourse.bass_utils import run_bass_kernel_spmd
    nc, in_maps = prepare(**inputs)
    res = run_bass_kernel_spmd(nc, in_maps, core_ids=list(range(NCORES)))

    grid_out = np.concatenate(
        [res.results[c]["grid_out"] for c in range(NCORES)], axis=0)
    mesh_parts = []
    for c in range(NCORES):
        n_real = NPC if c < NCORES - 1 else NM - (NCORES - 1) * NPC
        mesh_parts.append(res.results[c]["mesh_out"][:n_real])
    mesh_out = np.concatenate(mesh_parts, axis=0)
    return (grid_out, mesh_out)
